# revision 23
# baseline (speedup 1.0000x reference)
"""Trainium2 Bass kernel for a dense GAT layer (B=4, N=2048, FIN=128, K=4 heads, D=32).

Math (per batch b):
    Wh = (H @ W).reshape(N, K, D)
    s[i,k] = <Wh[i,k,:], a_src[k,:]>;  t[j,k] = <Wh[j,k,:], a_dst[k,:]>
    e[i,j,k] = leaky_relu(s[i,k] + t[j,k], 0.2), masked to -inf where A[i,j] == 0
    alpha = softmax_j(e);  out[i] = sum_j alpha[i,j,k] * Wh[j,k,:]

Reformulation (exact): with x = s_i + t_j,
    exp(lrelu(x)) = max(exp x, exp 0.2x); the i-side factor exp(0.2 s_i)
    cancels in the softmax.  With G = exp(0.8 s_i), HF_j = exp(t_j),
    F2_j = exp(0.2 t_j), m = (A > 0):
        v[j,i,k] = max(G_ik * HF_jk, F2_jk)          (= F2 * max(GH, 1))
        y[j,i,k] = v * m[j,i]
        num[i,k,:] = sum_j y * Wh[j,k,:];  den[i,k] = sum_j y
        out = num / den
    F2 folds into the tensor-scalar pass (HF as multiplier, F2 as max
    floor), so the PV stationary is raw [Wh | 1] and the denominator falls
    out of the ones column.  s and t come straight from H via host-side
    W@a_src / W@a_dst folds, so the score chain never waits on Wh.

All matmul operands and the score volume are bf16 (PE 1 cyc/row instead of
fp32's 4; DVE 4x mode on the tensor-scalar pass, 2x on the mask multiply).
The mask ships from the host as bf16 {0,1}.  Engine split per (jt, head) is
table-driven: DVE owns the mask TensorTensor (dominant pass) plus cheap 4x
TensorScalars, Pool takes one head's TensorScalar and some mask slices,
ACT absorbs one head via a relu+exp chain on broadcast s.

Sharding: 8 cores = 4 batches x 2 row-halves (i-slabs of 1024); no
collectives.  Host rotates H rows / A columns so each core's query rows are
local 0..1023 (identical SPMD program) and ships H pre-transposed.
"""

import numpy as np
import ml_dtypes
from contextlib import ExitStack

import concourse.bacc as bacc
import concourse.mybir as mybir
import concourse.tile as tile
from concourse.bass_utils import run_bass_kernel_spmd

B, N, FIN = 4, 2048, 128
KH, DH = 4, 32
P = 128
NI = 1024           # query rows per core
JT = N // P         # 16 j-chunks
WUC = 8 + 512       # warmup tile cols

f32 = mybir.dt.float32
bf16 = mybir.dt.bfloat16
BF = ml_dtypes.bfloat16

_CACHE = {}

# pass1 engine per (jt, k): 'v' = DVE TS, 'p' = Pool TS, 'a' = ACT relu+exp
PASS1_ENG = {}
for _jt in range(JT):
    PASS1_ENG[(_jt, 0)] = "v"
    PASS1_ENG[(_jt, 1)] = "p" if _jt % 4 == 0 else "v"
    PASS1_ENG[(_jt, 2)] = "p"
    PASS1_ENG[(_jt, 3)] = "v" if _jt < 3 else "a"
for _jt in (13, 15):
    PASS1_ENG[(_jt, 1)] = "a"
# jts whose k3 slice of the mask TT runs on Pool instead of DVE
TT_POOL_JT = {1, 4, 7, 10, 12, 14}

JW0 = 5   # initial warmup junk matmuls


def _build_program():
    nc = bacc.Bacc("TRN2", target_bir_lowering=False, debug=False)

    def din(name, shape, dtype):
        return nc.dram_tensor(name, list(shape), dtype, kind="ExternalInput").ap()

    CPW = P + 2 * KH + P + N              # [W | WSsrc | WSdst | identb | HT]
    cpack_d = din("cpack", (P, CPW), bf16)
    CP0 = 2 * P + 2 * KH                  # small head of cpack (no HT)
    mT_d = din("maskT", (N, NI), bf16)    # mask (A>0) transposed: [j, i]
    gscrG_d = nc.dram_tensor("gscrG", [KH, NI], bf16).ap()
    gscrS_d = nc.dram_tensor("gscrS", [KH, NI], bf16).ap()
    oaux_d = nc.dram_tensor("oaux", [2, 33, 4, 512], f32,
                            kind="ExternalOutput").ap()

    Exp = mybir.ActivationFunctionType.Exp
    Relu = mybir.ActivationFunctionType.Relu
    Copy = mybir.ActivationFunctionType.Copy
    MULT = mybir.AluOpType.mult
    MAX = mybir.AluOpType.max

    with tile.TileContext(nc) as tc, ExitStack() as ctx:
        const = ctx.enter_context(tc.tile_pool(name="const", bufs=1))
        big = ctx.enter_context(tc.tile_pool(name="big", bufs=1))
        vwork = ctx.enter_context(tc.tile_pool(name="vwork", bufs=5))
        ywork = ctx.enter_context(tc.tile_pool(name="ywork", bufs=4))
        small = ctx.enter_context(tc.tile_pool(name="small", bufs=2))
        ps = ctx.enter_context(tc.tile_pool(name="ps", bufs=1, space="PSUM"))
        pnp = ctx.enter_context(tc.tile_pool(name="pnp", bufs=2, space="PSUM"))
        pst = ctx.enter_context(tc.tile_pool(name="pst", bufs=1, space="PSUM"))
        pspv = ctx.enter_context(tc.tile_pool(name="pspv", bufs=1, space="PSUM"))

        # ---- inputs; DMA queue order is the critical schedule ----
        cpack = const.tile([P, CPW], bf16, tag="cpack")
        nc.sync.dma_start(cpack[:, 0:CP0], cpack_d[:, 0:CP0])
        nc.sync.dma_start(cpack[:, CP0:], cpack_d[:, CP0:])
        sbW = cpack[:, 0:P]
        wssrc = cpack[:, P:P + KH]
        wsdst = cpack[:, P + KH:P + 2 * KH]
        identb = cpack[:, P + 2 * KH:2 * P + 2 * KH]
        HT = cpack[:, 2 * P + 2 * KH:]  # [fin, n]

        maskT = big.tile([P, JT, NI], bf16, tag="maskT")
        for jt in range(2):  # first two mask chunks up front
            nc.sync.dma_start(maskT[:, jt, :], mT_d[jt * P:(jt + 1) * P, :])

        # PE warm-up junk on the small cpack head while HT lands
        for _ in range(JW0):
            pj = ps.tile([P, 512], f32, tag="stg")
            nc.tensor.matmul(pj[0:8, 0:256], cpack[:, 0:8], cpack[:, 8:8 + 256],
                             start=True, stop=True)

        # ---- srow/t straight from HT (host folded W into a_src/a_dst) ----
        GrowSb = small.tile([36, NI], bf16, tag="GrowSb", bufs=1)
        psrs = []
        for h in range(2):
            psr = ps.tile([P, 512], f32, tag="stg")
            nc.tensor.matmul(psr[0:KH, :], wssrc,
                             HT[:, h * 512:(h + 1) * 512],
                             start=True, stop=True)
            nc.scalar.activation(GrowSb[0:KH, h * 512:(h + 1) * 512],
                                 psr[0:KH, :], Exp, scale=0.8)
            psrs.append(psr)
        Gball = big.tile([P, KH, NI], bf16, tag="Gball")
        Sball = big.tile([P, 3, NI], bf16, tag="Sball")
        # head 0 broadcast on (idle) Pool: skips the DRAM round-trip
        nc.gpsimd.partition_broadcast(Gball[:, 0, :], GrowSb[0:1, :])
        nc.sync.dma_start(gscrG_d[:], GrowSb[0:KH, :])
        for k in [1, 2, 3]:
            nc.sync.dma_start(Gball[:, k, :], gscrG_d[k, :].partition_broadcast(P))
        for h in range(2):
            nc.scalar.copy(GrowSb[32:32 + KH, h * 512:(h + 1) * 512],
                           psrs[h][0:KH, :])
        nc.sync.dma_start(gscrS_d[:], GrowSb[32:32 + KH, :])
        nc.sync.dma_start(Sball[:, 2, :], gscrS_d[3, :].partition_broadcast(P))
        nc.sync.dma_start(Sball[:, 0, :], gscrS_d[1, :].partition_broadcast(P))
        nc.sync.dma_start(Sball[:, 1, :], gscrS_d[2, :].partition_broadcast(P))
        for jt in range(2, 5):
            nc.sync.dma_start(maskT[:, jt, :], mT_d[jt * P:(jt + 1) * P, :])
        # hold the bulk of the mask until the broadcast chain has the DMA
        # engine to itself (readiness-based arbitration would front-run it)
        with tc.tile_wait_until(0.013):
            for jt in range(5, JT):
                nc.sync.dma_start(maskT[:, jt, :], mT_d[jt * P:(jt + 1) * P, :])

        # t-scores for all j: ptt[j, jt*4+k]; HF = exp t, F2 = exp 0.2t
        ptt = pst.tile([P, JT * KH], f32, tag="ptt")
        for jt in range(JT):
            nc.tensor.matmul(ptt[:, jt * KH:(jt + 1) * KH],
                             HT[:, jt * P:(jt + 1) * P], wsdst,
                             start=True, stop=True, skip_group_check=True)
        HFcol = big.tile([P, JT * KH], f32, tag="HFcol")
        F2col = big.tile([P, JT * KH], f32, tag="F2col")
        tcol = big.tile([P, JT * KH], f32, tag="tcol")
        t02col = big.tile([P, JT * KH], f32, tag="t02col")
        nc.scalar.activation(HFcol[:], ptt[:], Exp, scale=1.0)
        nc.scalar.activation(F2col[:], ptt[:], Exp, scale=0.2)
        nc.scalar.copy(tcol[:], ptt[:])
        nc.scalar.activation(t02col[:], ptt[:], Copy, scale=0.2)

        # ---- whf[jt] = [Wh | 1] per head: Wh computed directly in [j, kd]
        # layout (stationary = HT chunk, moving = W); copies PSUM->SBUF are
        # split DVE/ACT; jts >= 4 are emitted inside the main loop.
        whf = []

        def emit_whf(jt):
            pn = pnp.tile([P, P], f32, tag="pn")
            nc.tensor.matmul(pn[:], HT[:, jt * P:(jt + 1) * P],
                             sbW, start=True, stop=True)
            wt = big.tile([P, KH, DH + 1], bf16, tag=f"whf{jt}", name=f"whf{jt}")
            nc.gpsimd.memset(wt[:, :, DH:DH + 1], 1.0)
            nc.scalar.copy(
                wt[:, :, 0:DH],
                pn[:].rearrange("p (k d) -> p k d", k=KH),
            )
            whf.append(wt)

        for jt in range(4):
            emit_whf(jt)

        # ---- main loop over j-chunks ----
        pv = [pspv.tile([97, 512], f32, tag=f"pv{q}", name=f"pv{q}")
              for q in range(4)]

        for jt in range(JT):
            if jt + 4 < JT:
                emit_whf(jt + 4)
            v = vwork.tile([P, KH, NI], bf16, tag="v")
            for k in range(KH):
                e = PASS1_ENG[(jt, k)]
                c = jt * KH + k
                if e == "a":
                    rt = small.tile([P, NI], bf16, tag="rt", bufs=3)
                    srow_idx = k - 1
                    nc.scalar.activation(rt[:], Sball[:, srow_idx, :], Relu,
                                         bias=tcol[:, c:c + 1], scale=1.0)
                    nc.scalar.activation(v[:, k, :], rt[:], Exp,
                                         bias=t02col[:, c:c + 1], scale=0.8)
                else:
                    eng = nc.vector if e == "v" else nc.gpsimd
                    eng.tensor_scalar(v[:, k, :], Gball[:, k, :],
                                      HFcol[:, c:c + 1], F2col[:, c:c + 1],
                                      MULT, MAX)
            y8 = ywork.tile([P, KH, NI], bf16, tag="y8")

            def pv_mm(k, ib):
                q, r = divmod(k, 2)
                isl = slice(ib * 512, (ib + 1) * 512)
                nc.tensor.matmul(
                    pv[q * 2 + ib][r * 64:r * 64 + 33, :],
                    whf[jt][:, k, :],
                    y8[:, k, isl],
                    start=(jt == 0), stop=(jt == JT - 1),
                    skip_group_check=True,
                )

            if jt < 3:
                # pipeline fill: per-head TT + PV as each broadcast lands
                for k in range(KH):
                    nc.vector.tensor_mul(y8[:, k], v[:, k], maskT[:, jt, :])
                    pv_mm(k, 0)
                    pv_mm(k, 1)
            else:
                if jt in TT_POOL_JT:
                    nc.vector.tensor_mul(
                        y8[:, 0:3], v[:, 0:3],
                        maskT[:, jt, None, :].broadcast_to((P, 3, NI)),
                    )
                    nc.gpsimd.tensor_mul(y8[:, 3], v[:, 3], maskT[:, jt, :])
                else:
                    nc.vector.tensor_mul(
                        y8[:], v[:],
                        maskT[:, jt, None, :].broadcast_to((P, KH, NI)),
                    )
                if jt == JT - 1:
                    for k in (0, 1):
                        pv_mm(k, 0)
                        pv_mm(k, 1)
                    for k in (2, 3):
                        pv_mm(k, 0)
                        pv_mm(k, 1)
                else:
                    for ib in range(2):
                        for k in range(KH):
                            pv_mm(k, ib)

        # ---- epilogue: raw accumulators out; host divides / transposes ----
        otall = small.tile([P, 4, 512], f32, tag="otall", bufs=1)
        for q in range(4):
            eng = nc.vector.tensor_copy if q % 2 == 0 else nc.scalar.copy
            eng(otall[0:33, q, :], pv[q][0:33, :])
        nc.sync.dma_start(oaux_d[0], otall[0:33, :, :])
        for q in range(4):
            eng = nc.vector.tensor_copy if q % 2 == 0 else nc.scalar.copy
            eng(otall[64:97, q, :], pv[q][64:97, :])
        nc.sync.dma_start(oaux_d[1], otall[64:97, :, :])

    nc.compile()
    return nc


def _host_prep(H, A, W, a_src, a_dst):
    """Build the 8 per-core input maps (layout prep + dtype casts only)."""
    Ssrc = np.zeros((FIN, KH), np.float32)
    Sdst = np.zeros((FIN, KH), np.float32)
    for k in range(KH):
        Ssrc[k * DH:(k + 1) * DH, k] = a_src[k]
        Sdst[k * DH:(k + 1) * DH, k] = a_dst[k]
    Wf = W.astype(np.float32)
    WSsrc = Wf @ Ssrc  # [FIN, KH]: s = H @ WSsrc
    WSdst = Wf @ Sdst

    in_maps = []
    for c in range(8):
        b, half = divmod(c, 2)
        i0 = half * NI
        HbT = np.roll(H[b], -i0, axis=0).T  # [FIN, N], j rolled
        maskT = np.ascontiguousarray(
            (np.roll(A[b, i0:i0 + NI, :], -i0, axis=1) > 0).T
        ).astype(BF)
        cpack = np.concatenate(
            [Wf, WSsrc, WSdst, np.eye(P, dtype=np.float32), HbT],
            axis=1,
        ).astype(BF)
        in_maps.append({
            "cpack": np.ascontiguousarray(cpack),
            "maskT": maskT,
        })
    return in_maps


def kernel(H, A, W, a_src, a_dst, _want_results=False, _trace=False):
    H = np.asarray(H); A = np.asarray(A); W = np.asarray(W)
    a_src = np.asarray(a_src); a_dst = np.asarray(a_dst)

    if "nc" not in _CACHE:
        _CACHE["nc"] = _build_program()
    nc = _CACHE["nc"]

    in_maps = _host_prep(H, A, W, a_src, a_dst)
    res = run_bass_kernel_spmd(nc, in_maps, list(range(8)), trace=_trace)

    out = np.empty((B, N, KH * DH), np.float32)
    for c in range(8):
        b, half = divmod(c, 2)
        i0 = half * NI
        aux = res.results[c]["oaux"]  # [2, 33, 4, 512] f32
        for q in range(4):
            p, ibb = divmod(q, 2)
            r0 = i0 + ibb * 512
            for h2 in range(2):
                k = 2 * p + h2
                blk = aux[h2, :, q, :]  # [33, 512]
                out[b, r0:r0 + 512, k * DH:(k + 1) * DH] = (
                    blk[0:DH] / blk[DH:DH + 1]
                ).T
    if _want_results:
        return out, res
    return out
